# revision 12
# baseline (speedup 1.0000x reference)
"""Trainium2 Bass kernel for a decoder layer (cross-attn + causal self-attn + FFN).

Sharding (8 NeuronCores):
  - Attention is head-parallel: core c owns heads {2c, 2c+1} over all B*S=4096
    tokens (batch flattened).
  - out_proj / layernorms / residual / FFN are sequence-parallel: core c owns
    tokens [512c, 512c+512).
  - Comm: AllToAll (head-concat -> token-shard) after each attention,
    AllGather (token-shard -> full) after LN2. No AllReduce anywhere.

Layouts: activations live transposed in SBUF ([feature, seq]) so every matmul
takes weights in natural [in, out] layout (lhsT) and activations as the moving
operand. LN gamma/beta are folded into consumer weights on the host. Matmul
inputs are bf16 (host-cast); residual stream and softmax statistics are f32.
"""

import sys
import numpy as np
import ml_dtypes
from contextlib import ExitStack

try:
    import concourse.bass as bass
except ImportError:  # pragma: no cover
    sys.path.insert(0, "/opt/trn_rl_repo")
    import concourse.bass as bass
import concourse.mybir as mybir
import concourse.tile as tile
from concourse.bass_utils import run_bass_kernel_spmd
from concourse.masks import make_identity

F32 = mybir.dt.float32
BF16 = mybir.dt.bfloat16
AX = mybir.AxisListType.X
ALU = mybir.AluOpType
ACTF = mybir.ActivationFunctionType
BF = ml_dtypes.bfloat16

NCORES = 8
B, S, D, H, FF = 2, 2048, 1024, 16, 4096
DH = D // H              # 64
T = B * S                # 4096 tokens (batch flattened)
SL = T // NCORES         # 512 tokens per core (seq shard)
HL = H // NCORES         # 2 heads per core
P = 128
DT = D // P              # 8 feature tiles
FFT = FF // P            # 32
QC = 512                 # q-chunk (tokens per attention column chunk)
NQ = S // QC             # 4 q chunks per batch
NKB = S // P             # 16 k blocks per batch
EPS = 1e-5
GROUP = [list(range(NCORES))]

_PROG = None
_LAST_EXEC_NS = None


def _legalize_waits(nc, limit=1):
    """Split multi-wait sync conditions onto same-engine NOPs.

    The walrus build in this container rejects engine instructions carrying
    more than one inline sync wait ("Too many sync wait commands"), so hoist
    the excess waits onto no-fuse NOPs inserted immediately before the
    instruction (same engine => executes in order, semantics preserved).
    """
    ENG = mybir.EngineType
    engines = {ENG.PE, ENG.DVE, ENG.Activation, ENG.Pool, ENG.SP}
    f = nc.m.functions[0]
    n_split = 0
    for b in f.blocks:
        il = b.instructions
        out = []
        for inst in il:
            si = getattr(inst, "sync_info", None)
            ow = list(si.on_wait) if si is not None and si.on_wait else []
            eng = getattr(inst, "engine", None)
            if eng in engines and len(ow) > limit:
                excess, keep = ow[:-limit], ow[-limit:]
                while excess:
                    chunk, excess = excess[:limit], excess[limit:]
                    ni = nc.engines[eng].nop(nofuse=True).ins
                    for bb in f.blocks:
                        try:
                            bb.instructions.remove(ni)
                            break
                        except ValueError:
                            pass
                    ni.sync_info = mybir.SyncInfo(on_wait=chunk, on_update=[])
                    out.append(ni)
                    n_split += 1
                inst.sync_info = mybir.SyncInfo(
                    on_wait=keep, on_update=si.on_update if si else [])
            out.append(inst)
        il[:] = out
    return n_split


# ---------------------------------------------------------------- program ---

def _ln_stats_normalize(nc, sp, t, tn, d_inner, eps_ap):
    """Natural-layout layernorm of one [128, d_inner] tile t -> tn (bf16)."""
    s1 = sp.tile([P, 1], F32, name="s1")
    nc.vector.reduce_sum(s1[:], t[:], axis=AX)
    sqs = sp.tile([P, d_inner], BF16, name="sqs")
    s2 = sp.tile([P, 1], F32, name="s2")
    nc.scalar.activation(sqs[:], t[:], ACTF.Square, accum_out=s2[:])
    neg_mu = sp.tile([P, 1], F32, name="neg_mu")
    nc.vector.tensor_scalar_mul(neg_mu[:], s1[:], -1.0 / d_inner)
    ex2 = sp.tile([P, 1], F32, name="ex2")
    nc.vector.tensor_scalar_mul(ex2[:], s2[:], 1.0 / d_inner)
    mu2 = sp.tile([P, 1], F32, name="mu2")
    nc.vector.tensor_tensor(mu2[:], neg_mu[:], neg_mu[:], op=ALU.mult)
    var = sp.tile([P, 1], F32, name="var")
    nc.vector.tensor_tensor(var[:], ex2[:], mu2[:], op=ALU.subtract)
    std = sp.tile([P, 1], F32, name="std")
    nc.scalar.activation(std[:], var[:], ACTF.Sqrt, bias=eps_ap)
    rstd = sp.tile([P, 1], F32, name="rstd")
    nc.vector.reciprocal(rstd[:], std[:])
    nc.vector.tensor_scalar(tn[:], t[:], neg_mu[:], rstd[:],
                            op0=ALU.add, op1=ALU.mult)


def _tp_block(nc, pp, dst_ap, src_ap, ident):
    """PE-transpose one 128x128 block src -> dst (via PSUM)."""
    ps = pp.tile([P, P], src_ap.dtype, name="tps")
    nc.tensor.transpose(ps[:], src_ap, ident)
    nc.vector.tensor_copy(dst_ap, ps[:])


def build_program():
    nc = bass.Bass(num_devices=NCORES)

    def din(name, shape, dt=BF16):
        return nc.declare_dram_parameter(name, list(shape), dt, isOutput=False)

    q_bf = din("q_bf", [T, D])
    m_bf = din("m_bf", [T, D])
    q_res = din("q_res", [SL, D], F32)
    ca_qw, ca_kw, ca_vw = din("ca_qw", [D, P]), din("ca_kw", [D, P]), din("ca_vw", [D, P])
    sa_qw, sa_kw, sa_vw = din("sa_qw", [D, P]), din("sa_kw", [D, P]), din("sa_vw", [D, P])
    ca_qb, ca_kb = din("ca_qb", [P, 1], F32), din("ca_kb", [P, 1], F32)
    sa_qb, sa_kb = din("sa_qb", [P, 1], F32), din("sa_kb", [P, 1], F32)
    ca_vb, sa_vb = din("ca_vb", [1, P]), din("sa_vb", [1, P])
    ca_ow, sa_ow = din("ca_ow", [D, D]), din("sa_ow", [D, D])
    ca_ob, sa_ob = din("ca_ob", [D, 1], F32), din("sa_ob", [D, 1], F32)
    w1, b1 = din("w1", [D, FF]), din("b1", [FF, 1], F32)
    w2, b2 = din("w2", [FF, D]), din("b2", [D, 1], F32)
    out_d = nc.declare_dram_parameter("out", [SL, D], F32, isOutput=True)

    with tile.TileContext(nc) as tc, ExitStack() as top:
        const = top.enter_context(tc.tile_pool(name="const", bufs=1))
        ident_bf = const.tile([P, P], BF16, name="ident_bf")
        make_identity(nc, ident_bf[:])
        ident_f32 = const.tile([P, P], F32, name="ident_f32")
        make_identity(nc, ident_f32[:])
        ones_1xP = const.tile([1, P], BF16, name="ones_1xP")
        nc.gpsimd.memset(ones_1xP[:], 1.0)
        ones_1x64 = const.tile([1, 64], BF16, name="ones_1x64")
        nc.gpsimd.memset(ones_1x64[:], 1.0)
        ones_Px1 = const.tile([P, 1], BF16, name="ones_Px1")
        nc.gpsimd.memset(ones_Px1[:], 1.0)
        eps_t = const.tile([P, 1], F32, name="eps_t")
        nc.gpsimd.memset(eps_t[:], EPS)

        # small bias tiles (live whole kernel)
        bias_pool = top.enter_context(tc.tile_pool(name="bias", bufs=1))

        def load_bias(name, dram, shape):
            t = bias_pool.tile(list(shape), F32, name=name)
            nc.sync.dma_start(out=t[:], in_=dram[:].rearrange(
                "(m p) o -> p (m o)", p=P) if shape[1] > 1 else dram[:])
            return t

        ca_qb_s = load_bias("ca_qb_s", ca_qb, [P, 1])
        ca_kb_s = load_bias("ca_kb_s", ca_kb, [P, 1])
        sa_qb_s = load_bias("sa_qb_s", sa_qb, [P, 1])
        sa_kb_s = load_bias("sa_kb_s", sa_kb, [P, 1])
        ca_ob_s = load_bias("ca_ob_s", ca_ob, [P, DT])
        sa_ob_s = load_bias("sa_ob_s", sa_ob, [P, DT])
        b1_s = load_bias("b1_s", b1, [P, FFT])
        b2_s = load_bias("b2_s", b2, [P, DT])
        ca_vb_s = bias_pool.tile([1, P], BF16, name="ca_vb_s")
        nc.sync.dma_start(out=ca_vb_s[:], in_=ca_vb[:])
        sa_vb_s = bias_pool.tile([1, P], BF16, name="sa_vb_s")
        nc.sync.dma_start(out=sa_vb_s[:], in_=sa_vb[:])

        resid = top.enter_context(tc.tile_pool(name="resid", bufs=1))
        xT0 = [resid.tile([P, SL], F32, name=f"xTa_{p}") for p in range(DT)]
        xT1 = [resid.tile([P, SL], F32, name=f"xTb_{p}") for p in range(DT)]

        # ---- transpose residual slice q_res -> xT0 (f32) ----
        with tc.tile_pool(name="qres_ld", bufs=2) as qp, \
             tc.tile_pool(name="qres_ps", bufs=4, space="PSUM") as pp:
            for i in range(SL // P):  # 4 row tiles
                t = qp.tile([P, D], F32, name="qres_t")
                nc.sync.dma_start(out=t[:], in_=q_res[i * P:(i + 1) * P, :])
                for p in range(DT):
                    _tp_block(nc, pp, xT0[p][:, i * P:(i + 1) * P],
                              t[:, p * P:(p + 1) * P], ident_f32[:])

        def attention(QTh, KTh, Vt, oTh, causal, scope_name):
            with ExitStack() as es:
                ep = es.enter_context(tc.tile_pool(name=f"{scope_name}_exp", bufs=6))
                sp = es.enter_context(tc.tile_pool(name=f"{scope_name}_sm", bufs=4))
                ps_s = es.enter_context(
                    tc.tile_pool(name=f"{scope_name}_pss", bufs=2, space="PSUM"))
                ps_o = es.enter_context(
                    tc.tile_pool(name=f"{scope_name}_pso", bufs=2, space="PSUM"))
                ps_b = es.enter_context(
                    tc.tile_pool(name=f"{scope_name}_psb", bufs=2, space="PSUM"))
                for b in range(B):
                    for t_ in range(HL):
                        for j in range(NQ):
                            q0 = b * S + j * QC
                            nkb = (j + 1) * (QC // P) if causal else NKB
                            pso = ps_o.tile([65, QC], F32, name=f"{scope_name}_o")
                            exps = []
                            for i in range(nkb):
                                pss = ps_s.tile([P, QC], F32, name=f"{scope_name}_s")
                                nc.tensor.matmul(
                                    pss[:],
                                    KTh[t_][:, b * S + i * P:b * S + (i + 1) * P],
                                    QTh[t_][:, q0:q0 + QC],
                                    start=True, stop=True)
                                ex = ep.tile([P, QC], BF16, name=f"{scope_name}_e")
                                nc.scalar.activation(ex[:], pss[:], ACTF.Exp,
                                                     scale=float(DH) ** -0.5)
                                if causal and i >= (QC // P) * j:
                                    # keep where q_global >= k_global
                                    nc.gpsimd.affine_select(
                                        out=ex[:], in_=ex[:],
                                        compare_op=ALU.is_ge, fill=0.0,
                                        base=j * QC - i * P,
                                        pattern=[[1, QC]], channel_multiplier=-1)
                                exps.append(ex)
                            for i in range(nkb):
                                nc.tensor.matmul(
                                    pso[:], Vt[b * NKB + i][:, 65 * t_:65 * t_ + 65],
                                    exps[i][:],
                                    start=(i == 0), stop=(i == nkb - 1))
                            rrow = sp.tile([1, QC], F32, name=f"{scope_name}_r")
                            nc.vector.reciprocal(rrow[:], pso[64:65, :])
                            rbf = sp.tile([1, QC], BF16, name=f"{scope_name}_rb")
                            nc.vector.tensor_copy(rbf[:], rrow[:])
                            psb = ps_b.tile([64, QC], F32, name=f"{scope_name}_b")
                            nc.tensor.matmul(psb[:], ones_1x64[:], rbf[:],
                                             start=True, stop=True)
                            ot = sp.tile([64, QC], BF16, name=f"{scope_name}_ot")
                            nc.vector.tensor_copy(ot[:], pso[0:64, :])
                            nc.vector.tensor_tensor(
                                oTh[t_][:, q0:q0 + QC],
                                ot[:], psb[:], op=ALU.mult)

        def a2a_outproj(oTh, ow_d, ob_s, x_in, x_out, scope_name):
            """AllToAll head-concat -> token shard, then out_proj + residual."""
            dram_es = ExitStack()
            dp = dram_es.enter_context(tc.tile_pool(
                name=f"{scope_name}_a2a_dram", bufs=1, space="DRAM"))
            a2a_in = dp.tile([NCORES * P, SL], BF16,
                             name=f"{scope_name}_a2a_in")
            a2a_out = dp.tile([NCORES * P, SL], BF16,
                              name=f"{scope_name}_a2a_out")
            for j in range(NCORES):
                for t_ in range(HL):
                    nc.sync.dma_start(
                        out=a2a_in[j * P + 64 * t_:j * P + 64 * t_ + 64, :],
                        in_=oTh[t_][:, j * SL:(j + 1) * SL])
            nc.gpsimd.collective_compute(
                "AllToAll", ALU.bypass, replica_groups=GROUP,
                ins=[a2a_in[:]], outs=[a2a_out[:]])
            with ExitStack() as es:
                rp = es.enter_context(tc.tile_pool(name=f"{scope_name}_rhs", bufs=1))
                wp = es.enter_context(tc.tile_pool(name=f"{scope_name}_oww", bufs=1))
                pp = es.enter_context(
                    tc.tile_pool(name=f"{scope_name}_psp", bufs=4, space="PSUM"))
                rhs = [rp.tile([P, SL], BF16, name=f"{scope_name}_rhs{c}")
                       for c in range(DT)]
                oww = [wp.tile([P, D], BF16, name=f"{scope_name}_ow{c}")
                       for c in range(DT)]
                for c in range(DT):
                    nc.sync.dma_start(out=rhs[c][:],
                                      in_=a2a_out[c * P:(c + 1) * P, :])
                    nc.sync.dma_start(out=oww[c][:], in_=ow_d[c * P:(c + 1) * P, :])
                for m in range(DT):
                    ps = pp.tile([P, SL], F32, name=f"{scope_name}_pso")
                    for c in range(DT):
                        nc.tensor.matmul(ps[:], oww[c][:, m * P:(m + 1) * P],
                                         rhs[c][:], start=(c == 0),
                                         stop=(c == DT - 1))
                    tb = rp.tile([P, SL], F32, name=f"{scope_name}_tb")
                    nc.vector.tensor_scalar_add(tb[:], ps[:], ob_s[:, m:m + 1])
                    nc.vector.tensor_tensor(x_out[m][:], tb[:], x_in[m][:],
                                            op=ALU.add)
            dram_es.close()

        def ln_T(x_in, out_bf, scope_name):
            """Transposed-layout LN over 8 feature tiles [128, SL] -> bf16."""
            with ExitStack() as es:
                sp = es.enter_context(tc.tile_pool(name=f"{scope_name}_sp", bufs=2))
                pp = es.enter_context(
                    tc.tile_pool(name=f"{scope_name}_ps", bufs=1, space="PSUM"))
                xb = [sp.tile([P, SL], BF16, name=f"{scope_name}_xb{p}")
                      for p in range(DT)]
                sq = [sp.tile([P, SL], BF16, name=f"{scope_name}_sq{p}")
                      for p in range(DT)]
                for p in range(DT):
                    nc.vector.tensor_copy(xb[p][:], x_in[p][:])
                    nc.vector.tensor_tensor(sq[p][:], xb[p][:], xb[p][:],
                                            op=ALU.mult)
                ps1 = pp.tile([1, SL], F32, name=f"{scope_name}_s1")
                ps2 = pp.tile([1, SL], F32, name=f"{scope_name}_s2")
                for p in range(DT):
                    nc.tensor.matmul(ps1[:], ones_Px1[:], xb[p][:],
                                     start=(p == 0), stop=(p == DT - 1))
                for p in range(DT):
                    nc.tensor.matmul(ps2[:], ones_Px1[:], sq[p][:],
                                     start=(p == 0), stop=(p == DT - 1))
                st = es.enter_context(tc.tile_pool(name=f"{scope_name}_st", bufs=1))
                neg_mu = st.tile([1, SL], F32, name=f"{scope_name}_nm")
                nc.vector.tensor_scalar_mul(neg_mu[:], ps1[:], -1.0 / D)
                ex2 = st.tile([1, SL], F32, name=f"{scope_name}_e2")
                nc.vector.tensor_scalar_mul(ex2[:], ps2[:], 1.0 / D)
                mu2 = st.tile([1, SL], F32, name=f"{scope_name}_m2")
                nc.vector.tensor_tensor(mu2[:], neg_mu[:], neg_mu[:], op=ALU.mult)
                var = st.tile([1, SL], F32, name=f"{scope_name}_va")
                nc.vector.tensor_tensor(var[:], ex2[:], mu2[:], op=ALU.subtract)
                std = st.tile([1, SL], F32, name=f"{scope_name}_sd")
                nc.scalar.activation(std[:], var[:], ACTF.Sqrt, bias=eps_t[0:1, 0:1])
                rstd = st.tile([1, SL], F32, name=f"{scope_name}_rs")
                nc.vector.reciprocal(rstd[:], std[:])
                nm_b = st.tile([1, SL], BF16, name=f"{scope_name}_nmb")
                nc.vector.tensor_copy(nm_b[:], neg_mu[:])
                rs_b = st.tile([1, SL], BF16, name=f"{scope_name}_rsb")
                nc.vector.tensor_copy(rs_b[:], rstd[:])
                pnm = pp.tile([P, SL], F32, name=f"{scope_name}_pnm")
                nc.tensor.matmul(pnm[:], ones_1xP[:], nm_b[:], start=True, stop=True)
                prs = pp.tile([P, SL], F32, name=f"{scope_name}_prs")
                nc.tensor.matmul(prs[:], ones_1xP[:], rs_b[:], start=True, stop=True)
                for p in range(DT):
                    tt = sp.tile([P, SL], F32, name=f"{scope_name}_tt")
                    nc.vector.tensor_tensor(tt[:], x_in[p][:], pnm[:], op=ALU.add)
                    nc.vector.tensor_tensor(out_bf[p][:], tt[:], prs[:],
                                            op=ALU.mult)

        # ================= Cross-attention block =================
        with ExitStack() as ca:
            big = ca.enter_context(tc.tile_pool(name="ca_big", bufs=1))
            QTh = [big.tile([64, T], BF16, name=f"ca_QT{t}") for t in range(HL)]
            KTh = [big.tile([64, T], BF16, name=f"ca_KT{t}") for t in range(HL)]
            Vt = [big.tile([P, 65 * HL], BF16, name=f"ca_V{i}")
                  for i in range(T // P)]
            oTh = [big.tile([64, T], BF16, name=f"ca_oT{t}") for t in range(HL)]

            # LN1(queries) -> transposed chunks, Q projection (chunk-wise)
            with tc.tile_pool(name="caq_w", bufs=1) as wp, \
                 tc.tile_pool(name="caq_ch", bufs=2) as chp, \
                 tc.tile_pool(name="caq_ld", bufs=3) as lp, \
                 tc.tile_pool(name="caq_sp", bufs=4) as sp, \
                 tc.tile_pool(name="caq_tps", bufs=3, space="PSUM") as tpp, \
                 tc.tile_pool(name="caq_ps", bufs=2, space="PSUM") as pp:
                qw_t = [wp.tile([P, P], BF16, name=f"caq_w{c}") for c in range(DT)]
                for c in range(DT):
                    nc.sync.dma_start(out=qw_t[c][:],
                                      in_=ca_qw[c * P:(c + 1) * P, :])
                for n in range(T // QC):
                    qch = [chp.tile([P, QC], BF16, name=f"qch{c}")
                           for c in range(DT)]
                    for ib in range(QC // P):
                        i = n * (QC // P) + ib
                        t = lp.tile([P, D], BF16, name="caq_t")
                        nc.sync.dma_start(out=t[:], in_=q_bf[i * P:(i + 1) * P, :])
                        tn = lp.tile([P, D], BF16, name="caq_tn")
                        _ln_stats_normalize(nc, sp, t, tn, D, eps_t[:, 0:1])
                        for c in range(DT):
                            _tp_block(nc, tpp, qch[c][:, ib * P:(ib + 1) * P],
                                      tn[:, c * P:(c + 1) * P], ident_bf[:])
                    ps = pp.tile([P, QC], F32, name="caq_ps")
                    for c in range(DT):
                        nc.tensor.matmul(ps[:], qw_t[c][:], qch[c][:],
                                         start=(c == 0), stop=(c == DT - 1))
                    for t_ in range(HL):
                        nc.vector.tensor_scalar_add(
                            QTh[t_][:, n * QC:(n + 1) * QC],
                            ps[64 * t_:64 * t_ + 64, :],
                            ca_qb_s[64 * t_:64 * t_ + 64, 0:1])

            # memory -> transposed chunks, K and V projections (chunk-wise)
            with tc.tile_pool(name="cakv_w", bufs=1) as wp, \
                 tc.tile_pool(name="cakv_ch", bufs=2) as chp, \
                 tc.tile_pool(name="cakv_ld", bufs=3) as lp, \
                 tc.tile_pool(name="cakv_tps", bufs=3, space="PSUM") as tpp, \
                 tc.tile_pool(name="cakv_ps", bufs=2, space="PSUM") as pp:
                kw_t = [wp.tile([P, P], BF16, name=f"cak_w{c}") for c in range(DT)]
                vw_t = [wp.tile([P, P], BF16, name=f"cav_w{c}") for c in range(DT)]
                for c in range(DT):
                    nc.sync.dma_start(out=kw_t[c][:],
                                      in_=ca_kw[c * P:(c + 1) * P, :])
                    nc.sync.dma_start(out=vw_t[c][:],
                                      in_=ca_vw[c * P:(c + 1) * P, :])
                for n in range(T // QC):
                    mch = [chp.tile([P, QC], BF16, name=f"mch{c}")
                           for c in range(DT)]
                    for ib in range(QC // P):
                        i = n * (QC // P) + ib
                        t = lp.tile([P, D], BF16, name="cakv_t")
                        nc.sync.dma_start(out=t[:], in_=m_bf[i * P:(i + 1) * P, :])
                        for c in range(DT):
                            _tp_block(nc, tpp, mch[c][:, ib * P:(ib + 1) * P],
                                      t[:, c * P:(c + 1) * P], ident_bf[:])
                    ps = pp.tile([P, QC], F32, name="cak_ps")
                    for c in range(DT):
                        nc.tensor.matmul(ps[:], kw_t[c][:], mch[c][:],
                                         start=(c == 0), stop=(c == DT - 1))
                    for t_ in range(HL):
                        nc.vector.tensor_scalar_add(
                            KTh[t_][:, n * QC:(n + 1) * QC],
                            ps[64 * t_:64 * t_ + 64, :],
                            ca_kb_s[64 * t_:64 * t_ + 64, 0:1])
                    for ib in range(QC // P):
                        i = n * (QC // P) + ib
                        psv = pp.tile([P, P], F32, name="cav_ps")
                        for c in range(DT):
                            nc.tensor.matmul(
                                psv[:], mch[c][:, ib * P:(ib + 1) * P],
                                vw_t[c][:], start=(c == 0), stop=False)
                        nc.tensor.matmul(psv[:], ones_1xP[:], ca_vb_s[:],
                                         start=False, stop=True)
                        for t_ in range(HL):
                            nc.vector.tensor_copy(
                                Vt[i][:, 65 * t_:65 * t_ + 64],
                                psv[:, 64 * t_:64 * t_ + 64])
                        for t_ in range(HL):
                            nc.gpsimd.memset(
                                Vt[i][:, 65 * t_ + 64:65 * t_ + 65], 1.0)

            attention(QTh, KTh, Vt, oTh, causal=False, scope_name="caat")
            a2a_outproj(oTh, ca_ow, ca_ob_s, xT0, xT1, "cao")

        # ================= Self-attention block =================
        with ExitStack() as sa:
            lnp = sa.enter_context(tc.tile_pool(name="ln2_out", bufs=1))
            ln2T = [lnp.tile([P, SL], BF16, name=f"ln2T{p}") for p in range(DT)]
            ln_T(xT1, ln2T, "ln2")
            dramp = sa.enter_context(
                tc.tile_pool(name="sa_ag_dram", bufs=1, space="DRAM"))
            ag_in = dramp.tile([D, SL], BF16, name="sa_ag_in")
            ag_out = dramp.tile([NCORES * D, SL], BF16, addr_space="Shared",
                                name="sa_ag_out")
            for p in range(DT):
                nc.sync.dma_start(out=ag_in[p * P:(p + 1) * P, :], in_=ln2T[p][:])
            nc.gpsimd.collective_compute(
                "AllGather", ALU.bypass, replica_groups=GROUP,
                ins=[ag_in[:]], outs=[ag_out[:]])

            big = sa.enter_context(tc.tile_pool(name="sa_big", bufs=1))
            QTh = [big.tile([64, T], BF16, name=f"sa_QT{t}") for t in range(HL)]
            KTh = [big.tile([64, T], BF16, name=f"sa_KT{t}") for t in range(HL)]
            Vt = [big.tile([P, 65 * HL], BF16, name=f"sa_V{i}")
                  for i in range(T // P)]
            oTh = [big.tile([64, T], BF16, name=f"sa_oT{t}") for t in range(HL)]

            with tc.tile_pool(name="sap_w", bufs=1) as wp, \
                 tc.tile_pool(name="sap_rhs", bufs=16) as rp, \
                 tc.tile_pool(name="sap_ps", bufs=3, space="PSUM") as pp:
                qw_t = [wp.tile([P, P], BF16, name=f"saq_w{c}") for c in range(DT)]
                kw_t = [wp.tile([P, P], BF16, name=f"sak_w{c}") for c in range(DT)]
                vw_t = [wp.tile([P, P], BF16, name=f"sav_w{c}") for c in range(DT)]
                for c in range(DT):
                    nc.sync.dma_start(out=qw_t[c][:], in_=sa_qw[c * P:(c + 1) * P, :])
                    nc.sync.dma_start(out=kw_t[c][:], in_=sa_kw[c * P:(c + 1) * P, :])
                    nc.sync.dma_start(out=vw_t[c][:], in_=sa_vw[c * P:(c + 1) * P, :])
                for n in range(T // QC):
                    rhs = []
                    for c in range(DT):
                        r = rp.tile([P, QC], BF16, name="sap_rhs")
                        nc.sync.dma_start(
                            out=r[:],
                            in_=ag_out[n * D + c * P:n * D + (c + 1) * P, :])
                        rhs.append(r)
                    for (w_t, b_s, dstT) in ((qw_t, sa_qb_s, QTh),
                                             (kw_t, sa_kb_s, KTh)):
                        ps = pp.tile([P, QC], F32, name="sap_psqk")
                        for c in range(DT):
                            nc.tensor.matmul(ps[:], w_t[c][:], rhs[c][:],
                                             start=(c == 0), stop=(c == DT - 1))
                        for t_ in range(HL):
                            nc.vector.tensor_scalar_add(
                                dstT[t_][:, n * QC:(n + 1) * QC],
                                ps[64 * t_:64 * t_ + 64, :],
                                b_s[64 * t_:64 * t_ + 64, 0:1])
                    for ib in range(QC // P):
                        i = n * (QC // P) + ib
                        psv = pp.tile([P, P], F32, name="sap_psv")
                        for c in range(DT):
                            nc.tensor.matmul(
                                psv[:], rhs[c][:, ib * P:(ib + 1) * P],
                                vw_t[c][:], start=(c == 0), stop=False)
                        nc.tensor.matmul(psv[:], ones_1xP[:], sa_vb_s[:],
                                         start=False, stop=True)
                        for t_ in range(HL):
                            nc.vector.tensor_copy(
                                Vt[i][:, 65 * t_:65 * t_ + 64],
                                psv[:, 64 * t_:64 * t_ + 64])
                        for t_ in range(HL):
                            nc.gpsimd.memset(
                                Vt[i][:, 65 * t_ + 64:65 * t_ + 65], 1.0)

            attention(QTh, KTh, Vt, oTh, causal=True, scope_name="saat")
            xT2 = [resid.tile([P, SL], F32, name=f"xTa_{p}") for p in range(DT)]
            a2a_outproj(oTh, sa_ow, sa_ob_s, xT1, xT2, "sao")

        # ================= FFN block =================
        with ExitStack() as ff:
            lnp = ff.enter_context(tc.tile_pool(name="ln3_out", bufs=1))
            ln3T = [lnp.tile([P, SL], BF16, name=f"ln3T{p}") for p in range(DT)]
            ln_T(xT2, ln3T, "ln3")
            h1p = ff.enter_context(tc.tile_pool(name="h1", bufs=1))
            h1 = [h1p.tile([P, SL], BF16, name=f"h1_{m}") for m in range(FFT)]
            with tc.tile_pool(name="w1p", bufs=2) as wp, \
                 tc.tile_pool(name="ff1_ps", bufs=4, space="PSUM") as pp:
                for mg in range(FF // QC):  # 8 groups of 4 m-tiles
                    w1p = []
                    for c in range(DT):
                        w = wp.tile([P, QC], BF16, name="w1p_t")
                        nc.sync.dma_start(
                            out=w[:],
                            in_=w1[c * P:(c + 1) * P, mg * QC:(mg + 1) * QC])
                        w1p.append(w)
                    for mm in range(QC // P):
                        m = mg * (QC // P) + mm
                        ps = pp.tile([P, SL], F32, name="ff1_ps")
                        for c in range(DT):
                            nc.tensor.matmul(ps[:],
                                             w1p[c][:, mm * P:(mm + 1) * P],
                                             ln3T[c][:], start=(c == 0),
                                             stop=(c == DT - 1))
                        nc.scalar.activation(h1[m][:], ps[:], ACTF.Gelu,
                                             bias=b1_s[:, m:m + 1])
            xTf = [resid.tile([P, SL], F32, name=f"xTb_{p}") for p in range(DT)]
            with tc.tile_pool(name="w2p", bufs=3) as wp, \
                 tc.tile_pool(name="ff2_ps", bufs=1, space="PSUM") as pp, \
                 tc.tile_pool(name="ff2_sb", bufs=4) as sp:
                psf = [pp.tile([P, SL], F32, name=f"ff2_ps{m}") for m in range(DT)]
                for c in range(FFT):
                    w = wp.tile([P, D], BF16, name="w2p_t")
                    nc.sync.dma_start(out=w[:], in_=w2[c * P:(c + 1) * P, :])
                    for m in range(DT):
                        nc.tensor.matmul(psf[m][:], w[:, m * P:(m + 1) * P],
                                         h1[c][:], start=(c == 0),
                                         stop=(c == FFT - 1))
                for m in range(DT):
                    tb = sp.tile([P, SL], F32, name="ff2_tb")
                    nc.vector.tensor_scalar_add(tb[:], psf[m][:], b2_s[:, m:m + 1])
                    nc.vector.tensor_tensor(xTf[m][:], tb[:], xT2[m][:],
                                            op=ALU.add)

        # ================= output transpose =================
        with tc.tile_pool(name="out_sb", bufs=2) as op, \
             tc.tile_pool(name="out_ps", bufs=4, space="PSUM") as pp:
            for sblk in range(SL // P):  # 4
                osb = op.tile([P, D], F32, name="out_t")
                for p in range(DT):
                    _tp_block(nc, pp, osb[:, p * P:(p + 1) * P],
                              xTf[p][:, sblk * P:(sblk + 1) * P], ident_f32[:])
                nc.sync.dma_start(out=out_d[sblk * P:(sblk + 1) * P, :],
                                  in_=osb[:])
    _legalize_waits(nc)
    return nc


# ------------------------------------------------------------------- host ---

def _prepare_in_maps(inputs):
    f32 = np.float32
    qs = np.ascontiguousarray(inputs["queries"].reshape(T, D)).astype(f32)
    ms = np.ascontiguousarray(inputs["memory"].reshape(T, D)).astype(f32)
    ln1_g, ln1_b = inputs["ln1_g"].astype(f32), inputs["ln1_b"].astype(f32)
    ln2_g, ln2_b = inputs["ln2_g"].astype(f32), inputs["ln2_b"].astype(f32)
    ln3_g, ln3_b = inputs["ln3_g"].astype(f32), inputs["ln3_b"].astype(f32)

    def fold(w, b, g, bl):
        w = w.astype(f32)
        return g[:, None] * w, b.astype(f32) + bl @ w

    ca_qw, ca_qb = fold(inputs["ca_qw"], inputs["ca_qb"], ln1_g, ln1_b)
    ca_kw, ca_kb = inputs["ca_kw"].astype(f32), inputs["ca_kb"].astype(f32)
    ca_vw, ca_vb = inputs["ca_vw"].astype(f32), inputs["ca_vb"].astype(f32)
    sa_qw, sa_qb = fold(inputs["sa_qw"], inputs["sa_qb"], ln2_g, ln2_b)
    sa_kw, sa_kb = fold(inputs["sa_kw"], inputs["sa_kb"], ln2_g, ln2_b)
    sa_vw, sa_vb = fold(inputs["sa_vw"], inputs["sa_vb"], ln2_g, ln2_b)
    w1, b1 = fold(inputs["w1"], inputs["b1"], ln3_g, ln3_b)
    w2, b2 = inputs["w2"].astype(f32), inputs["b2"].astype(f32)
    ca_ow, ca_ob = inputs["ca_ow"].astype(f32), inputs["ca_ob"].astype(f32)
    sa_ow, sa_ob = inputs["sa_ow"].astype(f32), inputs["sa_ob"].astype(f32)

    q_bf = qs.astype(BF)
    m_bf = ms.astype(BF)
    maps = []
    for c in range(NCORES):
        hs = slice(P * c, P * (c + 1))  # head-dim cols for heads {2c, 2c+1}
        maps.append({
            "q_bf": q_bf, "m_bf": m_bf,
            "q_res": np.ascontiguousarray(qs[SL * c:SL * (c + 1)]),
            "ca_qw": np.ascontiguousarray(ca_qw[:, hs]).astype(BF),
            "ca_kw": np.ascontiguousarray(ca_kw[:, hs]).astype(BF),
            "ca_vw": np.ascontiguousarray(ca_vw[:, hs]).astype(BF),
            "sa_qw": np.ascontiguousarray(sa_qw[:, hs]).astype(BF),
            "sa_kw": np.ascontiguousarray(sa_kw[:, hs]).astype(BF),
            "sa_vw": np.ascontiguousarray(sa_vw[:, hs]).astype(BF),
            "ca_qb": np.ascontiguousarray(ca_qb[hs])[:, None].astype(f32),
            "ca_kb": np.ascontiguousarray(ca_kb[hs])[:, None].astype(f32),
            "sa_qb": np.ascontiguousarray(sa_qb[hs])[:, None].astype(f32),
            "sa_kb": np.ascontiguousarray(sa_kb[hs])[:, None].astype(f32),
            "ca_vb": np.ascontiguousarray(ca_vb[hs])[None, :].astype(BF),
            "sa_vb": np.ascontiguousarray(sa_vb[hs])[None, :].astype(BF),
            "ca_ow": ca_ow.astype(BF), "sa_ow": sa_ow.astype(BF),
            "ca_ob": ca_ob[:, None].astype(f32),
            "sa_ob": sa_ob[:, None].astype(f32),
            "w1": w1.astype(BF), "b1": b1[:, None].astype(f32),
            "w2": w2.astype(BF), "b2": b2[:, None].astype(f32),
        })
    return maps


def kernel(**inputs) -> np.ndarray:
    global _PROG, _LAST_EXEC_NS
    import os
    if _PROG is None:
        _PROG = build_program()
    maps = _prepare_in_maps(inputs)
    trace = bool(int(os.environ.get("TRN_PROFILE", "0")))
    res = run_bass_kernel_spmd(_PROG, maps, list(range(NCORES)), trace=trace)
    _LAST_EXEC_NS = res.exec_time_ns
    out = np.concatenate([res.results[c]["out"] for c in range(NCORES)], axis=0)
    return out.reshape(B, S, D).astype(np.float32)


# revision 18
# speedup vs baseline: 46.6851x; 46.6851x over previous
"""Trainium2 Bass kernel for a decoder layer (cross-attn + causal self-attn + FFN).

Sharding (8 NeuronCores):
  - Attention is head-parallel: core c owns heads {2c, 2c+1} over all B*S=4096
    tokens (batch flattened).
  - out_proj / layernorms / residual / FFN are sequence-parallel: core c owns
    tokens [512c, 512c+512).
  - Comm: AllToAll (head-concat -> token-shard) after each attention,
    AllGather (token-shard -> full) after LN2. No AllReduce anywhere.

Layouts: activations live transposed in SBUF ([feature, seq]) so every matmul
takes weights in natural [in, out] layout (lhsT) and activations as the moving
operand. LN gamma/beta are folded into consumer weights on the host. Matmul
inputs are bf16 (host-cast); residual stream and softmax statistics are f32.
"""

import sys
import numpy as np
import ml_dtypes
from contextlib import ExitStack

try:
    import concourse.bass as bass
except ImportError:  # pragma: no cover
    sys.path.insert(0, "/opt/trn_rl_repo")
    import concourse.bass as bass
import concourse.mybir as mybir
import concourse.tile as tile
from concourse.bass_utils import run_bass_kernel_spmd
from concourse.masks import make_identity

F32 = mybir.dt.float32
BF16 = mybir.dt.bfloat16
AX = mybir.AxisListType.X
ALU = mybir.AluOpType
ACTF = mybir.ActivationFunctionType
BF = ml_dtypes.bfloat16

NCORES = 8
B, S, D, H, FF = 2, 2048, 1024, 16, 4096
DH = D // H              # 64
T = B * S                # 4096 tokens (batch flattened)
SL = T // NCORES         # 512 tokens per core (seq shard)
HL = H // NCORES         # 2 heads per core
P = 128
DT = D // P              # 8 feature tiles
FFT = FF // P            # 32
QC = 512                 # q-chunk (tokens per attention column chunk)
NQ = S // QC             # 4 q chunks per batch
NKB = S // P             # 16 k blocks per batch
EPS = 1e-5
GROUP = [list(range(NCORES))]

_PROG = None
_LAST_EXEC_NS = None


def _legalize_waits(nc, limit=1):
    """Split multi-wait sync conditions onto same-engine NOPs.

    The walrus build in this container rejects engine instructions carrying
    more than one inline sync wait ("Too many sync wait commands"), so hoist
    the excess waits onto no-fuse NOPs inserted immediately before the
    instruction (same engine => executes in order, semantics preserved).
    """
    ENG = mybir.EngineType
    engines = {ENG.PE, ENG.DVE, ENG.Activation, ENG.Pool, ENG.SP}
    f = nc.m.functions[0]
    n_split = 0
    for b in f.blocks:
        il = b.instructions
        out = []
        for inst in il:
            si = getattr(inst, "sync_info", None)
            ow = list(si.on_wait) if si is not None and si.on_wait else []
            eng = getattr(inst, "engine", None)
            if eng in engines and len(ow) > limit:
                excess, keep = ow[:-limit], ow[-limit:]
                while excess:
                    chunk, excess = excess[:limit], excess[limit:]
                    ni = nc.engines[eng].nop(nofuse=True).ins
                    for bb in f.blocks:
                        try:
                            bb.instructions.remove(ni)
                            break
                        except ValueError:
                            pass
                    ni.sync_info = mybir.SyncInfo(on_wait=chunk, on_update=[])
                    out.append(ni)
                    n_split += 1
                inst.sync_info = mybir.SyncInfo(
                    on_wait=keep, on_update=si.on_update if si else [])
            out.append(inst)
        il[:] = out
    return n_split


# ---------------------------------------------------------------- program ---

def build_program(stop_after=None):
    # stop_after in {"ca_proj", "ca_attn", "ca_out", "ln2ag", "sa_proj",
    #                "sa_attn", "sa_out", "ffn1", None}
    nc = bass.Bass(num_devices=NCORES)

    def din(name, shape, dt=BF16):
        return nc.declare_dram_parameter(name, list(shape), dt, isOutput=False)

    qT_bf = din("qT_bf", [D, T])
    mT_bf = din("mT_bf", [D, T])
    q_resT = din("q_resT", [D, SL], F32)
    ca_qw, ca_kw, ca_vw = din("ca_qw", [D, P]), din("ca_kw", [D, P]), din("ca_vw", [D, P])
    sa_qw, sa_kw, sa_vw = din("sa_qw", [D, P]), din("sa_kw", [D, P]), din("sa_vw", [D, P])
    ca_qb, ca_kb = din("ca_qb", [P, 1], F32), din("ca_kb", [P, 1], F32)
    sa_qb, sa_kb = din("sa_qb", [P, 1], F32), din("sa_kb", [P, 1], F32)
    ca_vb, sa_vb = din("ca_vb", [1, P]), din("sa_vb", [1, P])
    ca_ow, sa_ow = din("ca_ow", [D, D]), din("sa_ow", [D, D])
    ca_ob, sa_ob = din("ca_ob", [D, 1], F32), din("sa_ob", [D, 1], F32)
    w1, b1 = din("w1", [D, FF]), din("b1", [FF, 1], F32)
    w2, b2 = din("w2", [FF, D]), din("b2", [D, 1], F32)
    out_d = nc.declare_dram_parameter("out", [D, SL], F32, isOutput=True)

    with tile.TileContext(nc) as tc, ExitStack() as top:
        const = top.enter_context(tc.tile_pool(name="const", bufs=1))
        # Causal-mask additive bias tiles for the 4 diagonal-straddling
        # (kblock, qchunk) offsets: mask[x, y] = 0 if (y + 128*r - x) >= 0
        # else -1e9, where r = i - 4*j in 0..3 (k_global = 128i+x,
        # q_global = 512j+y).
        mask_t = []
        for r in range(QC // P):
            mt = const.tile([P, QC], F32, name=f"mask_{r}")
            nc.gpsimd.memset(mt[:], 0.0)
            nc.gpsimd.affine_select(
                out=mt[:], in_=mt[:], compare_op=ALU.is_ge, fill=-1e9,
                base=-P * r, pattern=[[1, QC]], channel_multiplier=-1)
            mask_t.append(mt)
        ones_1xP = const.tile([1, P], BF16, name="ones_1xP")
        nc.gpsimd.memset(ones_1xP[:], 1.0)
        ones_1x64 = const.tile([1, 64], BF16, name="ones_1x64")
        nc.gpsimd.memset(ones_1x64[:], 1.0)
        ones_Px1 = const.tile([P, 1], BF16, name="ones_Px1")
        nc.gpsimd.memset(ones_Px1[:], 1.0)
        eps_t = const.tile([P, 1], F32, name="eps_t")
        nc.gpsimd.memset(eps_t[:], EPS)

        # small bias tiles (live whole kernel)
        bias_pool = top.enter_context(tc.tile_pool(name="bias", bufs=1))

        def load_bias(name, dram, shape):
            t = bias_pool.tile(list(shape), F32, name=name)
            nc.sync.dma_start(out=t[:], in_=dram[:].rearrange(
                "(m p) o -> p (m o)", p=P) if shape[1] > 1 else dram[:])
            return t

        ca_qb_s = load_bias("ca_qb_s", ca_qb, [P, 1])
        ca_kb_s = load_bias("ca_kb_s", ca_kb, [P, 1])
        sa_qb_s = load_bias("sa_qb_s", sa_qb, [P, 1])
        sa_kb_s = load_bias("sa_kb_s", sa_kb, [P, 1])
        ca_ob_s = load_bias("ca_ob_s", ca_ob, [P, DT])
        sa_ob_s = load_bias("sa_ob_s", sa_ob, [P, DT])
        b1_s = load_bias("b1_s", b1, [P, FFT])
        b2_s = load_bias("b2_s", b2, [P, DT])
        ca_vb_s = bias_pool.tile([1, P], BF16, name="ca_vb_s")
        nc.sync.dma_start(out=ca_vb_s[:], in_=ca_vb[:])
        sa_vb_s = bias_pool.tile([1, P], BF16, name="sa_vb_s")
        nc.sync.dma_start(out=sa_vb_s[:], in_=sa_vb[:])

        resid = top.enter_context(tc.tile_pool(name="resid", bufs=1))
        xT0 = [resid.tile([P, SL], F32, name=f"xTa_{p}") for p in range(DT)]
        xT1 = [resid.tile([P, SL], F32, name=f"xTb_{p}") for p in range(DT)]

        # ---- residual slice (host-transposed) -> xT0 (f32) ----
        for p in range(DT):
            nc.sync.dma_start(out=xT0[p][:],
                              in_=q_resT[p * P:(p + 1) * P, :])

        def attention(QTh, KTh, Vt, oTh, causal, scope_name):
            with ExitStack() as es:
                ep = es.enter_context(tc.tile_pool(name=f"{scope_name}_exp", bufs=6))
                sp = es.enter_context(tc.tile_pool(name=f"{scope_name}_sm", bufs=4))
                ps_s = es.enter_context(
                    tc.tile_pool(name=f"{scope_name}_pss", bufs=2, space="PSUM"))
                ps_o = es.enter_context(
                    tc.tile_pool(name=f"{scope_name}_pso", bufs=2, space="PSUM"))
                ps_b = es.enter_context(
                    tc.tile_pool(name=f"{scope_name}_psb", bufs=2, space="PSUM"))
                for b in range(B):
                    for t_ in range(HL):
                        for j in range(NQ):
                            q0 = b * S + j * QC
                            nkb = (j + 1) * (QC // P) if causal else NKB
                            pso = ps_o.tile([65, QC], F32, name=f"{scope_name}_o")
                            exps = []
                            for i in range(nkb):
                                pss = ps_s.tile([P, QC], F32, name=f"{scope_name}_s")
                                nc.tensor.matmul(
                                    pss[:],
                                    KTh[t_][:, b * S + i * P:b * S + (i + 1) * P],
                                    QTh[t_][:, q0:q0 + QC],
                                    start=True, stop=True)
                                if causal and i >= (QC // P) * j:
                                    nc.vector.tensor_tensor(
                                        pss[:], pss[:],
                                        mask_t[i - (QC // P) * j][:], op=ALU.add)
                                ex = ep.tile([P, QC], BF16, name=f"{scope_name}_e")
                                nc.scalar.activation(ex[:], pss[:], ACTF.Exp,
                                                     scale=float(DH) ** -0.5)
                                exps.append(ex)
                            for i in range(nkb):
                                nc.tensor.matmul(
                                    pso[:], Vt[b * NKB + i][:, 65 * t_:65 * t_ + 65],
                                    exps[i][:],
                                    start=(i == 0), stop=(i == nkb - 1))
                            rrow = sp.tile([1, QC], F32, name=f"{scope_name}_r")
                            nc.vector.reciprocal(rrow[:], pso[64:65, :])
                            rbf = sp.tile([1, QC], BF16, name=f"{scope_name}_rb")
                            nc.vector.tensor_copy(rbf[:], rrow[:])
                            psb = ps_b.tile([64, QC], F32, name=f"{scope_name}_b")
                            nc.tensor.matmul(psb[:], ones_1x64[:], rbf[:],
                                             start=True, stop=True)
                            ot = sp.tile([64, QC], BF16, name=f"{scope_name}_ot")
                            nc.vector.tensor_copy(ot[:], pso[0:64, :])
                            nc.vector.tensor_tensor(
                                oTh[t_][:, q0:q0 + QC],
                                ot[:], psb[:], op=ALU.mult)

        def a2a_outproj(oTh, ow_d, ob_s, x_in, x_out, scope_name):
            """AllToAll head-concat -> token shard, then out_proj + residual."""
            dram_es = ExitStack()
            dp = dram_es.enter_context(tc.tile_pool(
                name=f"{scope_name}_a2a_dram", bufs=1, space="DRAM"))
            a2a_in = dp.tile([NCORES * P, SL], BF16,
                             name=f"{scope_name}_a2a_in")
            a2a_out = dp.tile([NCORES * P, SL], BF16,
                              name=f"{scope_name}_a2a_out")
            for j in range(NCORES):
                for t_ in range(HL):
                    nc.sync.dma_start(
                        out=a2a_in[j * P + 64 * t_:j * P + 64 * t_ + 64, :],
                        in_=oTh[t_][:, j * SL:(j + 1) * SL])
            nc.gpsimd.collective_compute(
                "AllToAll", ALU.bypass, replica_groups=GROUP,
                ins=[a2a_in[:]], outs=[a2a_out[:]])
            with ExitStack() as es:
                rp = es.enter_context(tc.tile_pool(name=f"{scope_name}_rhs", bufs=1))
                wp = es.enter_context(tc.tile_pool(name=f"{scope_name}_oww", bufs=1))
                pp = es.enter_context(
                    tc.tile_pool(name=f"{scope_name}_psp", bufs=4, space="PSUM"))
                rhs = [rp.tile([P, SL], BF16, name=f"{scope_name}_rhs{c}")
                       for c in range(DT)]
                oww = [wp.tile([P, D], BF16, name=f"{scope_name}_ow{c}")
                       for c in range(DT)]
                for c in range(DT):
                    nc.sync.dma_start(out=rhs[c][:],
                                      in_=a2a_out[c * P:(c + 1) * P, :])
                    nc.sync.dma_start(out=oww[c][:], in_=ow_d[c * P:(c + 1) * P, :])
                for m in range(DT):
                    ps = pp.tile([P, SL], F32, name=f"{scope_name}_pso")
                    for c in range(DT):
                        nc.tensor.matmul(ps[:], oww[c][:, m * P:(m + 1) * P],
                                         rhs[c][:], start=(c == 0),
                                         stop=(c == DT - 1))
                    tb = rp.tile([P, SL], F32, name=f"{scope_name}_tb")
                    nc.vector.tensor_scalar_add(tb[:], ps[:], ob_s[:, m:m + 1])
                    nc.vector.tensor_tensor(x_out[m][:], tb[:], x_in[m][:],
                                            op=ALU.add)
            dram_es.close()

        def ln_T(x_in, out_bf, scope_name):
            """Transposed-layout LN over 8 feature tiles [128, SL] -> bf16."""
            with ExitStack() as es:
                sp = es.enter_context(tc.tile_pool(name=f"{scope_name}_sp", bufs=2))
                pp = es.enter_context(
                    tc.tile_pool(name=f"{scope_name}_ps", bufs=1, space="PSUM"))
                xb = [sp.tile([P, SL], BF16, name=f"{scope_name}_xb{p}")
                      for p in range(DT)]
                sq = [sp.tile([P, SL], BF16, name=f"{scope_name}_sq{p}")
                      for p in range(DT)]
                for p in range(DT):
                    nc.vector.tensor_copy(xb[p][:], x_in[p][:])
                    nc.vector.tensor_tensor(sq[p][:], xb[p][:], xb[p][:],
                                            op=ALU.mult)
                ps1 = pp.tile([1, SL], F32, name=f"{scope_name}_s1")
                ps2 = pp.tile([1, SL], F32, name=f"{scope_name}_s2")
                for p in range(DT):
                    nc.tensor.matmul(ps1[:], ones_Px1[:], xb[p][:],
                                     start=(p == 0), stop=(p == DT - 1))
                for p in range(DT):
                    nc.tensor.matmul(ps2[:], ones_Px1[:], sq[p][:],
                                     start=(p == 0), stop=(p == DT - 1))
                st = es.enter_context(tc.tile_pool(name=f"{scope_name}_st", bufs=1))
                neg_mu = st.tile([1, SL], F32, name=f"{scope_name}_nm")
                nc.vector.tensor_scalar_mul(neg_mu[:], ps1[:], -1.0 / D)
                ex2 = st.tile([1, SL], F32, name=f"{scope_name}_e2")
                nc.vector.tensor_scalar_mul(ex2[:], ps2[:], 1.0 / D)
                mu2 = st.tile([1, SL], F32, name=f"{scope_name}_m2")
                nc.vector.tensor_tensor(mu2[:], neg_mu[:], neg_mu[:], op=ALU.mult)
                var = st.tile([1, SL], F32, name=f"{scope_name}_va")
                nc.vector.tensor_tensor(var[:], ex2[:], mu2[:], op=ALU.subtract)
                std = st.tile([1, SL], F32, name=f"{scope_name}_sd")
                nc.scalar.activation(std[:], var[:], ACTF.Sqrt, bias=eps_t[0:1, 0:1])
                rstd = st.tile([1, SL], F32, name=f"{scope_name}_rs")
                nc.vector.reciprocal(rstd[:], std[:])
                nm_b = st.tile([1, SL], BF16, name=f"{scope_name}_nmb")
                nc.vector.tensor_copy(nm_b[:], neg_mu[:])
                rs_b = st.tile([1, SL], BF16, name=f"{scope_name}_rsb")
                nc.vector.tensor_copy(rs_b[:], rstd[:])
                pnm = pp.tile([P, SL], F32, name=f"{scope_name}_pnm")
                nc.tensor.matmul(pnm[:], ones_1xP[:], nm_b[:], start=True, stop=True)
                prs = pp.tile([P, SL], F32, name=f"{scope_name}_prs")
                nc.tensor.matmul(prs[:], ones_1xP[:], rs_b[:], start=True, stop=True)
                for p in range(DT):
                    tt = sp.tile([P, SL], F32, name=f"{scope_name}_tt")
                    nc.vector.tensor_tensor(tt[:], x_in[p][:], pnm[:], op=ALU.add)
                    nc.vector.tensor_tensor(out_bf[p][:], tt[:], prs[:],
                                            op=ALU.mult)

        # ================= Cross-attention block =================
        with ExitStack() as ca:
            big = ca.enter_context(tc.tile_pool(name="ca_big", bufs=1))
            QTh = [big.tile([64, T], BF16, name=f"ca_QT{t}") for t in range(HL)]
            KTh = [big.tile([64, T], BF16, name=f"ca_KT{t}") for t in range(HL)]
            Vt = [big.tile([P, 65 * HL], BF16, name=f"ca_V{i}")
                  for i in range(T // P)]
            oTh = [big.tile([64, T], BF16, name=f"ca_oT{t}") for t in range(HL)]

            # LN1 (transposed layout, chunk-wise) + Q projection
            with tc.tile_pool(name="caq_w", bufs=1) as wp, \
                 tc.tile_pool(name="caq_ch", bufs=2) as chp, \
                 tc.tile_pool(name="caq_sq", bufs=2) as sqp, \
                 tc.tile_pool(name="caq_st", bufs=2) as stp, \
                 tc.tile_pool(name="caq_sps", bufs=1, space="PSUM") as spp, \
                 tc.tile_pool(name="caq_bps", bufs=1, space="PSUM") as bpp, \
                 tc.tile_pool(name="caq_ps", bufs=2, space="PSUM") as pp:
                qw_t = [wp.tile([P, P], BF16, name=f"caq_w{c}") for c in range(DT)]
                for c in range(DT):
                    nc.sync.dma_start(out=qw_t[c][:],
                                      in_=ca_qw[c * P:(c + 1) * P, :])
                for n in range(T // QC):
                    qt = [chp.tile([P, QC], BF16, name=f"qt{c}")
                          for c in range(DT)]
                    sq = [sqp.tile([P, QC], BF16, name=f"qsq{c}")
                          for c in range(DT)]
                    for c in range(DT):
                        nc.sync.dma_start(
                            out=qt[c][:],
                            in_=qT_bf[c * P:(c + 1) * P, n * QC:(n + 1) * QC])
                        nc.vector.tensor_tensor(sq[c][:], qt[c][:], qt[c][:],
                                                op=ALU.mult)
                    ps1 = spp.tile([1, QC], F32, name="caq_s1")
                    ps2 = spp.tile([1, QC], F32, name="caq_s2")
                    for c in range(DT):
                        nc.tensor.matmul(ps1[:], ones_Px1[:], qt[c][:],
                                         start=(c == 0), stop=(c == DT - 1))
                    for c in range(DT):
                        nc.tensor.matmul(ps2[:], ones_Px1[:], sq[c][:],
                                         start=(c == 0), stop=(c == DT - 1))
                    neg_mu = stp.tile([1, QC], F32, name="caq_nm")
                    nc.vector.tensor_scalar_mul(neg_mu[:], ps1[:], -1.0 / D)
                    ex2 = stp.tile([1, QC], F32, name="caq_e2")
                    nc.vector.tensor_scalar_mul(ex2[:], ps2[:], 1.0 / D)
                    mu2 = stp.tile([1, QC], F32, name="caq_m2")
                    nc.vector.tensor_tensor(mu2[:], neg_mu[:], neg_mu[:],
                                            op=ALU.mult)
                    var = stp.tile([1, QC], F32, name="caq_va")
                    nc.vector.tensor_tensor(var[:], ex2[:], mu2[:],
                                            op=ALU.subtract)
                    std = stp.tile([1, QC], F32, name="caq_sd")
                    nc.scalar.activation(std[:], var[:], ACTF.Sqrt,
                                         bias=eps_t[0:1, 0:1])
                    rstd = stp.tile([1, QC], F32, name="caq_rs")
                    nc.vector.reciprocal(rstd[:], std[:])
                    nm_b = stp.tile([1, QC], BF16, name="caq_nmb")
                    nc.vector.tensor_copy(nm_b[:], neg_mu[:])
                    rs_b = stp.tile([1, QC], BF16, name="caq_rsb")
                    nc.vector.tensor_copy(rs_b[:], rstd[:])
                    pnm = bpp.tile([P, QC], F32, name="caq_pnm")
                    nc.tensor.matmul(pnm[:], ones_1xP[:], nm_b[:],
                                     start=True, stop=True)
                    prs = bpp.tile([P, QC], F32, name="caq_prs")
                    nc.tensor.matmul(prs[:], ones_1xP[:], rs_b[:],
                                     start=True, stop=True)
                    qch = [chp.tile([P, QC], BF16, name=f"qch{c}")
                           for c in range(DT)]
                    for c in range(DT):
                        t1 = sqp.tile([P, QC], F32, name="caq_t1")
                        nc.vector.tensor_tensor(t1[:], qt[c][:], pnm[:],
                                                op=ALU.add)
                        nc.vector.tensor_tensor(qch[c][:], t1[:], prs[:],
                                                op=ALU.mult)
                    ps = pp.tile([P, QC], F32, name="caq_ps")
                    for c in range(DT):
                        nc.tensor.matmul(ps[:], qw_t[c][:], qch[c][:],
                                         start=(c == 0), stop=(c == DT - 1))
                    for t_ in range(HL):
                        nc.vector.tensor_scalar_add(
                            QTh[t_][:, n * QC:(n + 1) * QC],
                            ps[64 * t_:64 * t_ + 64, :],
                            ca_qb_s[64 * t_:64 * t_ + 64, 0:1])

            # memory (host-transposed, raw) -> K and V projections
            with tc.tile_pool(name="cakv_w", bufs=1) as wp, \
                 tc.tile_pool(name="cakv_ch", bufs=2) as chp, \
                 tc.tile_pool(name="cakv_ps", bufs=2, space="PSUM") as pp, \
                 tc.tile_pool(name="cakv_psv", bufs=2, space="PSUM") as pvp:
                kw_t = [wp.tile([P, P], BF16, name=f"cak_w{c}") for c in range(DT)]
                vw_t = [wp.tile([P, P], BF16, name=f"cav_w{c}") for c in range(DT)]
                for c in range(DT):
                    nc.sync.dma_start(out=kw_t[c][:],
                                      in_=ca_kw[c * P:(c + 1) * P, :])
                    nc.sync.dma_start(out=vw_t[c][:],
                                      in_=ca_vw[c * P:(c + 1) * P, :])
                for n in range(T // QC):
                    mch = [chp.tile([P, QC], BF16, name=f"mch{c}")
                           for c in range(DT)]
                    for c in range(DT):
                        nc.sync.dma_start(
                            out=mch[c][:],
                            in_=mT_bf[c * P:(c + 1) * P, n * QC:(n + 1) * QC])
                    ps = pp.tile([P, QC], F32, name="cak_ps")
                    for c in range(DT):
                        nc.tensor.matmul(ps[:], kw_t[c][:], mch[c][:],
                                         start=(c == 0), stop=(c == DT - 1))
                    for t_ in range(HL):
                        nc.vector.tensor_scalar_add(
                            KTh[t_][:, n * QC:(n + 1) * QC],
                            ps[64 * t_:64 * t_ + 64, :],
                            ca_kb_s[64 * t_:64 * t_ + 64, 0:1])
                    for ib in range(QC // P):
                        i = n * (QC // P) + ib
                        psv = pvp.tile([P, P], F32, name="cav_ps")
                        for c in range(DT):
                            nc.tensor.matmul(
                                psv[:], mch[c][:, ib * P:(ib + 1) * P],
                                vw_t[c][:], start=(c == 0), stop=False)
                        nc.tensor.matmul(psv[:], ones_1xP[:], ca_vb_s[:],
                                         start=False, stop=True)
                        for t_ in range(HL):
                            nc.vector.tensor_copy(
                                Vt[i][:, 65 * t_:65 * t_ + 64],
                                psv[:, 64 * t_:64 * t_ + 64])
                        for t_ in range(HL):
                            nc.gpsimd.memset(
                                Vt[i][:, 65 * t_ + 64:65 * t_ + 65], 1.0)

            if stop_after == "ca_proj":
                _legalize_waits(nc)
                return nc
            attention(QTh, KTh, Vt, oTh, causal=False, scope_name="caat")
            if stop_after == "ca_attn":
                _legalize_waits(nc)
                return nc
            a2a_outproj(oTh, ca_ow, ca_ob_s, xT0, xT1, "cao")
            if stop_after == "ca_out":
                _legalize_waits(nc)
                return nc

        # ================= Self-attention block =================
        with ExitStack() as sa:
            lnp = sa.enter_context(tc.tile_pool(name="ln2_out", bufs=1))
            ln2T = [lnp.tile([P, SL], BF16, name=f"ln2T{p}") for p in range(DT)]
            ln_T(xT1, ln2T, "ln2")
            dramp = sa.enter_context(
                tc.tile_pool(name="sa_ag_dram", bufs=1, space="DRAM"))
            ag_in = dramp.tile([D, SL], BF16, name="sa_ag_in")
            ag_out = dramp.tile([NCORES * D, SL], BF16, addr_space="Shared",
                                name="sa_ag_out")
            for p in range(DT):
                nc.sync.dma_start(out=ag_in[p * P:(p + 1) * P, :], in_=ln2T[p][:])
            nc.gpsimd.collective_compute(
                "AllGather", ALU.bypass, replica_groups=GROUP,
                ins=[ag_in[:]], outs=[ag_out[:]])
            if stop_after == "ln2ag":
                _legalize_waits(nc)
                return nc

            big = sa.enter_context(tc.tile_pool(name="sa_big", bufs=1))
            QTh = [big.tile([64, T], BF16, name=f"sa_QT{t}") for t in range(HL)]
            KTh = [big.tile([64, T], BF16, name=f"sa_KT{t}") for t in range(HL)]
            Vt = [big.tile([P, 65 * HL], BF16, name=f"sa_V{i}")
                  for i in range(T // P)]
            oTh = [big.tile([64, T], BF16, name=f"sa_oT{t}") for t in range(HL)]

            with tc.tile_pool(name="sap_w", bufs=1) as wp, \
                 tc.tile_pool(name="sap_rhs", bufs=16) as rp, \
                 tc.tile_pool(name="sap_ps", bufs=3, space="PSUM") as pp:
                qw_t = [wp.tile([P, P], BF16, name=f"saq_w{c}") for c in range(DT)]
                kw_t = [wp.tile([P, P], BF16, name=f"sak_w{c}") for c in range(DT)]
                vw_t = [wp.tile([P, P], BF16, name=f"sav_w{c}") for c in range(DT)]
                for c in range(DT):
                    nc.sync.dma_start(out=qw_t[c][:], in_=sa_qw[c * P:(c + 1) * P, :])
                    nc.sync.dma_start(out=kw_t[c][:], in_=sa_kw[c * P:(c + 1) * P, :])
                    nc.sync.dma_start(out=vw_t[c][:], in_=sa_vw[c * P:(c + 1) * P, :])
                for n in range(T // QC):
                    rhs = []
                    for c in range(DT):
                        r = rp.tile([P, QC], BF16, name="sap_rhs")
                        nc.sync.dma_start(
                            out=r[:],
                            in_=ag_out[n * D + c * P:n * D + (c + 1) * P, :])
                        rhs.append(r)
                    for (w_t, b_s, dstT) in ((qw_t, sa_qb_s, QTh),
                                             (kw_t, sa_kb_s, KTh)):
                        ps = pp.tile([P, QC], F32, name="sap_psqk")
                        for c in range(DT):
                            nc.tensor.matmul(ps[:], w_t[c][:], rhs[c][:],
                                             start=(c == 0), stop=(c == DT - 1))
                        for t_ in range(HL):
                            nc.vector.tensor_scalar_add(
                                dstT[t_][:, n * QC:(n + 1) * QC],
                                ps[64 * t_:64 * t_ + 64, :],
                                b_s[64 * t_:64 * t_ + 64, 0:1])
                    for ib in range(QC // P):
                        i = n * (QC // P) + ib
                        psv = pp.tile([P, P], F32, name="sap_psv")
                        for c in range(DT):
                            nc.tensor.matmul(
                                psv[:], rhs[c][:, ib * P:(ib + 1) * P],
                                vw_t[c][:], start=(c == 0), stop=False)
                        nc.tensor.matmul(psv[:], ones_1xP[:], sa_vb_s[:],
                                         start=False, stop=True)
                        for t_ in range(HL):
                            nc.vector.tensor_copy(
                                Vt[i][:, 65 * t_:65 * t_ + 64],
                                psv[:, 64 * t_:64 * t_ + 64])
                        for t_ in range(HL):
                            nc.gpsimd.memset(
                                Vt[i][:, 65 * t_ + 64:65 * t_ + 65], 1.0)

            if stop_after == "sa_proj":
                _legalize_waits(nc)
                return nc
            attention(QTh, KTh, Vt, oTh, causal=True, scope_name="saat")
            if stop_after == "sa_attn":
                _legalize_waits(nc)
                return nc
            xT2 = [resid.tile([P, SL], F32, name=f"xTa_{p}") for p in range(DT)]
            a2a_outproj(oTh, sa_ow, sa_ob_s, xT1, xT2, "sao")
            if stop_after == "sa_out":
                _legalize_waits(nc)
                return nc

        # ================= FFN block =================
        with ExitStack() as ff:
            lnp = ff.enter_context(tc.tile_pool(name="ln3_out", bufs=1))
            ln3T = [lnp.tile([P, SL], BF16, name=f"ln3T{p}") for p in range(DT)]
            ln_T(xT2, ln3T, "ln3")
            h1p = ff.enter_context(tc.tile_pool(name="h1", bufs=1))
            h1 = [h1p.tile([P, SL], BF16, name=f"h1_{m}") for m in range(FFT)]
            with tc.tile_pool(name="w1p", bufs=2) as wp, \
                 tc.tile_pool(name="ff1_ps", bufs=4, space="PSUM") as pp:
                for mg in range(FF // QC):  # 8 groups of 4 m-tiles
                    w1p = []
                    for c in range(DT):
                        w = wp.tile([P, QC], BF16, name="w1p_t")
                        nc.sync.dma_start(
                            out=w[:],
                            in_=w1[c * P:(c + 1) * P, mg * QC:(mg + 1) * QC])
                        w1p.append(w)
                    for mm in range(QC // P):
                        m = mg * (QC // P) + mm
                        ps = pp.tile([P, SL], F32, name="ff1_ps")
                        for c in range(DT):
                            nc.tensor.matmul(ps[:],
                                             w1p[c][:, mm * P:(mm + 1) * P],
                                             ln3T[c][:], start=(c == 0),
                                             stop=(c == DT - 1))
                        nc.scalar.activation(h1[m][:], ps[:], ACTF.Gelu,
                                             bias=b1_s[:, m:m + 1])
            xTf = [resid.tile([P, SL], F32, name=f"xTb_{p}") for p in range(DT)]
            if stop_after == "ffn1":
                _legalize_waits(nc)
                return nc
            with tc.tile_pool(name="w2p", bufs=3) as wp, \
                 tc.tile_pool(name="ff2_ps", bufs=1, space="PSUM") as pp, \
                 tc.tile_pool(name="ff2_sb", bufs=4) as sp:
                psf = [pp.tile([P, SL], F32, name=f"ff2_ps{m}") for m in range(DT)]
                for c in range(FFT):
                    w = wp.tile([P, D], BF16, name="w2p_t")
                    nc.sync.dma_start(out=w[:], in_=w2[c * P:(c + 1) * P, :])
                    for m in range(DT):
                        nc.tensor.matmul(psf[m][:], w[:, m * P:(m + 1) * P],
                                         h1[c][:], start=(c == 0),
                                         stop=(c == FFT - 1))
                for m in range(DT):
                    tb = sp.tile([P, SL], F32, name="ff2_tb")
                    nc.vector.tensor_scalar_add(tb[:], psf[m][:], b2_s[:, m:m + 1])
                    nc.vector.tensor_tensor(xTf[m][:], tb[:], xT2[m][:],
                                            op=ALU.add)

        # ================= output (transposed; host untransposes) =========
        for p in range(DT):
            nc.sync.dma_start(out=out_d[p * P:(p + 1) * P, :], in_=xTf[p][:])
    _legalize_waits(nc)
    return nc


# ------------------------------------------------------------------- host ---

def _prepare_in_maps(inputs):
    f32 = np.float32
    qs = np.ascontiguousarray(inputs["queries"].reshape(T, D)).astype(f32)
    ms = np.ascontiguousarray(inputs["memory"].reshape(T, D)).astype(f32)
    ln1_g, ln1_b = inputs["ln1_g"].astype(f32), inputs["ln1_b"].astype(f32)
    ln2_g, ln2_b = inputs["ln2_g"].astype(f32), inputs["ln2_b"].astype(f32)
    ln3_g, ln3_b = inputs["ln3_g"].astype(f32), inputs["ln3_b"].astype(f32)

    def fold(w, b, g, bl):
        w = w.astype(f32)
        return g[:, None] * w, b.astype(f32) + bl @ w

    ca_qw, ca_qb = fold(inputs["ca_qw"], inputs["ca_qb"], ln1_g, ln1_b)
    ca_kw, ca_kb = inputs["ca_kw"].astype(f32), inputs["ca_kb"].astype(f32)
    ca_vw, ca_vb = inputs["ca_vw"].astype(f32), inputs["ca_vb"].astype(f32)
    sa_qw, sa_qb = fold(inputs["sa_qw"], inputs["sa_qb"], ln2_g, ln2_b)
    sa_kw, sa_kb = fold(inputs["sa_kw"], inputs["sa_kb"], ln2_g, ln2_b)
    sa_vw, sa_vb = fold(inputs["sa_vw"], inputs["sa_vb"], ln2_g, ln2_b)
    w1, b1 = fold(inputs["w1"], inputs["b1"], ln3_g, ln3_b)
    w2, b2 = inputs["w2"].astype(f32), inputs["b2"].astype(f32)
    ca_ow, ca_ob = inputs["ca_ow"].astype(f32), inputs["ca_ob"].astype(f32)
    sa_ow, sa_ob = inputs["sa_ow"].astype(f32), inputs["sa_ob"].astype(f32)

    qT_bf = np.ascontiguousarray(qs.T).astype(BF)
    mT_bf = np.ascontiguousarray(ms.T).astype(BF)
    qsT = np.ascontiguousarray(qs.T)
    maps = []
    for c in range(NCORES):
        hs = slice(P * c, P * (c + 1))  # head-dim cols for heads {2c, 2c+1}
        maps.append({
            "qT_bf": qT_bf, "mT_bf": mT_bf,
            "q_resT": np.ascontiguousarray(qsT[:, SL * c:SL * (c + 1)]),
            "ca_qw": np.ascontiguousarray(ca_qw[:, hs]).astype(BF),
            "ca_kw": np.ascontiguousarray(ca_kw[:, hs]).astype(BF),
            "ca_vw": np.ascontiguousarray(ca_vw[:, hs]).astype(BF),
            "sa_qw": np.ascontiguousarray(sa_qw[:, hs]).astype(BF),
            "sa_kw": np.ascontiguousarray(sa_kw[:, hs]).astype(BF),
            "sa_vw": np.ascontiguousarray(sa_vw[:, hs]).astype(BF),
            "ca_qb": np.ascontiguousarray(ca_qb[hs])[:, None].astype(f32),
            "ca_kb": np.ascontiguousarray(ca_kb[hs])[:, None].astype(f32),
            "sa_qb": np.ascontiguousarray(sa_qb[hs])[:, None].astype(f32),
            "sa_kb": np.ascontiguousarray(sa_kb[hs])[:, None].astype(f32),
            "ca_vb": np.ascontiguousarray(ca_vb[hs])[None, :].astype(BF),
            "sa_vb": np.ascontiguousarray(sa_vb[hs])[None, :].astype(BF),
            "ca_ow": ca_ow.astype(BF), "sa_ow": sa_ow.astype(BF),
            "ca_ob": ca_ob[:, None].astype(f32),
            "sa_ob": sa_ob[:, None].astype(f32),
            "w1": w1.astype(BF), "b1": b1[:, None].astype(f32),
            "w2": w2.astype(BF), "b2": b2[:, None].astype(f32),
        })
    return maps


def kernel(**inputs) -> np.ndarray:
    global _PROG, _LAST_EXEC_NS
    import os
    if _PROG is None:
        _PROG = build_program()
    maps = _prepare_in_maps(inputs)
    trace = bool(int(os.environ.get("TRN_PROFILE", "0")))
    res = run_bass_kernel_spmd(_PROG, maps, list(range(NCORES)), trace=trace)
    _LAST_EXEC_NS = res.exec_time_ns
    out = np.concatenate([res.results[c]["out"].T for c in range(NCORES)],
                         axis=0)
    return out.reshape(B, S, D).astype(np.float32)
